# revision 33
# baseline (speedup 1.0000x reference)
"""Trainium2 Bass kernel for nn_CConv (causal depthwise FFT-conv, 512 taps).

The reference's FFT conv is exactly a causal depthwise conv1d with a
512-step learned init state prepended:
    out[b,t,c] = sum_k f[k,c] * xc[b, t+512-k, c],  xc = concat(init, x)

Mapping to the tensor engine (per channel c):
    out[i, (m0,b)] = sum_q  W_q[j,i].T @ X[j, (m0+q, b)]
where W_q[j,i] = f[i-j+128*(4-q), c] are 5 Toeplitz slices of a
[128 x 640] band, and X is the natural time-tiled x (partition = t%128,
columns = (time chunk, batch)).  The 5 matmuls accumulate in PSUM.
Channels are sharded 8 ways across cores; inside a core, channels
stream in groups of 8 (x slab + band rectangles -> 40 matmuls ->
PSUM->SBUF cast -> batched output DMA).

HBM traffic is the bottleneck, so: operands are fp16 (filter prescaled
by 32, x by 1/32 — exact powers of two keeping both far from fp16
subnormals; fp32 PSUM accumulation), the output is written fp16 and
upcast on host, and the Toeplitz band's zero triangles are never
shipped: band tiles live in a persistent 4-slot ring that is zeroed
once, and each group's DMAs write only the nonzero 32-row-block
rectangles band[32R:32R+32, 32R:32R+544] (85% of the full band).
"""

import os

import numpy as np

import concourse.bacc as bacc
import concourse.mybir as mybir
from concourse.bass_utils import run_bass_kernel_spmd
from concourse.tile import TileContext

B, L, D, CLEN = 4, 4096, 1024, 512
NCORES = 8
DSH = D // NCORES            # 128 channels per core
GCH = 8                      # channels per group
NG = DSH // GCH              # 16 groups per core
MIN = (CLEN + L) // 128      # 36 input chunks (4 init + 32 x)
MOUT = L // 128              # 32 output blocks
NQ = 5                       # contraction chunks per output block
XW = MIN * B                 # 144 x columns per channel
BW = 640                     # band columns per channel
NR = 4                       # 32-row blocks per band
RW = 544                     # nonzero band columns per 32-row block
NRING = 4                    # band ring depth
SCALE = 32.0

_CACHE = {}
LAST_RESULTS = None          # BassKernelResults of the most recent run


def _build_bass():
    # Bacc (not plain Bass): its compile() legalizes sync waits (>1 wait per
    # instruction gets split into InstEventSemaphore), which walrus requires.
    nc = bacc.Bacc(None, target_bir_lowering=False)
    f16, f32 = mybir.dt.float16, mybir.dt.float32
    xd = nc.declare_dram_parameter("xs", [NG, 128, GCH * XW], f16, isOutput=False)
    # one [32, GCH, 544] block per group: the 4 diagonal row-blocks of the
    # band are identical content, so the 4 DMAs re-read the same region
    bd = nc.declare_dram_parameter("br", [NG, 32, GCH, RW], f16, isOutput=False)
    od = nc.declare_dram_parameter("out", [NG, 128, GCH, 128], f16, isOutput=True)

    with TileContext(nc) as tc:
        with (
            tc.tile_pool(name="bring", bufs=1) as bp,
            tc.tile_pool(name="sp", bufs=4) as sp,
            tc.tile_pool(name="op", bufs=4) as op,
            tc.tile_pool(name="pp", bufs=4, space="PSUM") as pp,
        ):
            ring = []
            for r in range(NRING):
                bt = bp.tile([128, GCH, BW], f16, tag=f"band{r}")
                nc.vector.memset(bt[:], 0.0)
                ring.append(bt)
            batches = [(0, 4), (4, 4), (8, 4), (12, 2), (14, 1), (15, 1)]
            for g0, nb in batches:
                ot = op.tile([128, nb, GCH, 128], f16)
                for k in range(nb):
                    g = g0 + k
                    st = sp.tile([128, GCH * XW], f16)
                    nc.sync.dma_start(out=st[:], in_=xd[g])
                    bt = ring[g % NRING]
                    for r in range(NR):
                        nc.sync.dma_start(
                            out=bt[32 * r : 32 * r + 32, :, 32 * r : 32 * r + RW],
                            in_=bd[g],
                        )
                    for c in range(GCH):
                        ps = pp.tile([128, 128], f32)
                        for q in range(NQ):
                            nc.tensor.matmul(
                                ps[:],
                                lhsT=bt[:, c, 128 * (4 - q) : 128 * (5 - q)],
                                rhs=st[:, c * XW + 4 * q : c * XW + 4 * q + 128],
                                start=(q == 0),
                                stop=(q == NQ - 1),
                            )
                        nc.vector.tensor_copy(out=ot[:, k, c, :], in_=ps[:])
                nc.sync.dma_start(
                    out=od[g0 : g0 + nb].transpose([1, 0, 2, 3]), in_=ot[:]
                )
    nc.finalize()  # Bacc.compile(): reg alloc + sync-wait legalization
    return nc


def _prep_inputs(x, last_input_init, filt):
    """Host-side: cast/scale to fp16 and prearrange into the exact SBUF
    layout so every DMA is a contiguous line-rate copy.

    xs[core, g, j, c*144 + m*4 + b] = xc[b, 128*m + j, ch] / 32
    br[core, g, r, j', c, w]        = 32*f[(32r + w) - (32r + j'), ch]
                                    = band[ch, 32r + j', 32r + w]
    """
    x = np.asarray(x, dtype=np.float32)
    init = np.asarray(last_input_init, dtype=np.float32)
    filt = np.asarray(filt, dtype=np.float32)

    xc = np.concatenate(
        [np.broadcast_to(init[None], (B, CLEN, D)), x], axis=1
    )  # [B, 4608, D]
    xh = (xc * np.float32(1.0 / SCALE)).astype(np.float16)
    xr = xh.reshape(B, MIN, 128, D)                      # [b, m, j, ch]
    xt = xr.transpose(3, 2, 1, 0)                        # [ch, j, m, b]
    xt = np.ascontiguousarray(xt).reshape(D, 128, XW)    # [ch, j, m*4+b]
    xg = xt.reshape(NCORES, NG, GCH, 128, XW)
    xs = np.ascontiguousarray(
        xg.transpose(0, 1, 3, 2, 4).reshape(NCORES, NG, 128, GCH * XW)
    )

    fs = (filt * np.float32(SCALE)).astype(np.float16)   # [512, D]
    pf = np.zeros((D, 767), np.float16)
    pf[:, 127:639] = fs.T
    jj = np.arange(32)
    ww = np.arange(RW)
    # within a 32-row block the diagonal offset is the same for every r:
    # band[ch, 32r + j', 32r + w] = pf[ch, 127 - j' + w]
    idx = 127 - jj[:, None] + ww[None, :]                # [32, 544]
    blk = pf[:, idx]                                     # [ch, j', w] shared by all r
    bg = blk.reshape(NCORES, NG, GCH, 32, RW)
    br = np.ascontiguousarray(bg.transpose(0, 1, 3, 2, 4))
    return xs, br


def kernel(x, last_input_init, filt):
    global LAST_RESULTS
    if "nc" not in _CACHE:
        _CACHE["nc"] = _build_bass()
    nc = _CACHE["nc"]

    xs, br = _prep_inputs(x, last_input_init, filt)
    in_maps = [{"xs": xs[core], "br": br[core]} for core in range(NCORES)]

    trace = bool(os.environ.get("BASS_TRACE"))
    res = run_bass_kernel_spmd(nc, in_maps, list(range(NCORES)), trace=trace)
    LAST_RESULTS = res

    outs = []
    for core in range(NCORES):
        o = res.results[core]["out"].astype(np.float32)  # [NG, 128, GCH, 128]
        o = o.reshape(NG, 128, GCH, MOUT, B)             # [g, i, c, m0, b]
        o = o.transpose(4, 3, 1, 0, 2)                   # [b, m0, i, g, c]
        outs.append(o.reshape(B, L, DSH))
    out = np.concatenate(outs, axis=2)
    return np.ascontiguousarray(out, dtype=np.float32)


# revision 34
# speedup vs baseline: 1.4730x; 1.4730x over previous
"""Trainium2 Bass kernel for nn_CConv (causal depthwise FFT-conv, 512 taps).

The reference's FFT conv is exactly a causal depthwise conv1d with a
512-step learned init state prepended:
    out[b,t,c] = sum_k f[k,c] * xc[b, t+512-k, c],  xc = concat(init, x)

Mapping to the tensor engine (per channel c):
    out[i, (m0,b)] = sum_q  W_q[j,i].T @ X[j, (m0+q, b)]
where W_q[j,i] = f[i-j+128*(4-q), c] are 5 Toeplitz slices of a
[128 x 640] band built on the host, and X is the natural time-tiled
x (partition = t%128, columns = (time chunk, batch)).  The 5 matmuls
accumulate in PSUM.  Channels are sharded 8 ways across cores; inside a
core, channels stream in groups of 8 (fused x+band slab DMA -> 40
matmuls -> PSUM->SBUF cast -> batched output DMA).

HBM traffic is the bottleneck (the profile shows the DMA stream at
~355 GB/s, 99% of the per-core HBM roofline), so: operands are fp16
(filter prescaled by 32, x by 1/32 — exact powers of two keeping both
far from fp16 subnormals; fp32 PSUM accumulation) and the output is
written fp16 and upcast on host.  All DMAs are contiguous host-
prearranged slabs; output DMAs are batched 4 groups at a time with
smaller final batches so the kernel tail isn't gated on a large DMA.

Variants tried and rejected (kept out of the final kernel):
- on-chip Toeplitz expansion via SWDGE diagonal-AP DMAs: correct but
  SWDGE per-DMA overhead + single-port source reads made it slower;
- shipping only the nonzero 32-row-block rectangles of the band into a
  persistent zeroed ring: the ring memsets and quarter-partition DMAs
  cost more than the 3MB of HBM they saved;
- fp32/float32r matmuls (4x/4x cycles per row at N=128), TB=64 tiling
  (halves band bytes but doubles PE streaming time), FFT-by-matmul
  (4-8x the FLOPs).
"""

import os

import numpy as np

import concourse.bacc as bacc
import concourse.mybir as mybir
from concourse.bass_utils import run_bass_kernel_spmd
from concourse.tile import TileContext

B, L, D, CLEN = 4, 4096, 1024, 512
NCORES = 8
DSH = D // NCORES            # 128 channels per core
GCH = 8                      # channels per group
NG = DSH // GCH              # 16 groups per core
MIN = (CLEN + L) // 128      # 36 input chunks (4 init + 32 x)
MOUT = L // 128              # 32 output blocks
NQ = 5                       # contraction chunks per output block
XW = MIN * B                 # 144 x columns per channel
BW = 640                     # band columns per channel
SLABW = GCH * (XW + BW)      # 6272 slab columns per partition row
SCALE = 32.0

_CACHE = {}
LAST_RESULTS = None          # BassKernelResults of the most recent run


def _build_bass():
    # Bacc (not plain Bass): its compile() legalizes sync waits (>1 wait per
    # instruction gets split into InstEventSemaphore), which walrus requires.
    nc = bacc.Bacc(None, target_bir_lowering=False)
    f16, f32 = mybir.dt.float16, mybir.dt.float32
    sd = nc.declare_dram_parameter("slab", [NG, 128, SLABW], f16, isOutput=False)
    od = nc.declare_dram_parameter("out", [NG, 128, GCH, 128], f16, isOutput=True)

    with TileContext(nc) as tc:
        with (
            tc.tile_pool(name="sp", bufs=4) as sp,
            tc.tile_pool(name="op", bufs=4) as op,
            tc.tile_pool(name="pp", bufs=4, space="PSUM") as pp,
        ):
            batches = [(0, 4), (4, 4), (8, 4), (12, 2), (14, 1), (15, 1)]
            for g0, nb in batches:
                ot = op.tile([128, nb, GCH, 128], f16)
                for k in range(nb):
                    g = g0 + k
                    st = sp.tile([128, SLABW], f16)
                    if g == 0:
                        nc.sync.dma_start(
                            out=st[:, : 2 * XW], in_=sd[g, :, : 2 * XW]
                        )
                        nc.sync.dma_start(
                            out=st[:, 2 * XW :], in_=sd[g, :, 2 * XW :]
                        )
                    else:
                        nc.sync.dma_start(out=st[:], in_=sd[g])
                    for c in range(GCH):
                        ps = pp.tile([128, 128], f32)
                        xo = c * XW
                        bo = GCH * XW + c * BW
                        for q in range(NQ):
                            nc.tensor.matmul(
                                ps[:],
                                lhsT=st[:, bo + 128 * (4 - q) : bo + 128 * (5 - q)],
                                rhs=st[:, xo + 4 * q : xo + 4 * q + 128],
                                start=(q == 0),
                                stop=(q == NQ - 1),
                            )
                        nc.vector.tensor_copy(out=ot[:, k, c, :], in_=ps[:])
                nc.sync.dma_start(
                    out=od[g0 : g0 + nb].transpose([1, 0, 2, 3]), in_=ot[:]
                )
    nc.finalize()  # Bacc.compile(): reg alloc + sync-wait legalization
    return nc


def _prep_inputs(x, last_input_init, filt):
    """Host-side: cast/scale to fp16 and prearrange into the exact SBUF
    layout so every DMA is a contiguous line-rate copy.

    slab[core, g, j, c*144 + m*4 + b]          = xc[b, 128*m + j, ch] / 32
    slab[core, g, j, 1152 + c*640 + u]         = 32*f[u - j, ch]
    """
    x = np.asarray(x, dtype=np.float32)
    init = np.asarray(last_input_init, dtype=np.float32)
    filt = np.asarray(filt, dtype=np.float32)

    xc = np.concatenate(
        [np.broadcast_to(init[None], (B, CLEN, D)), x], axis=1
    )  # [B, 4608, D]
    xh = (xc * np.float32(1.0 / SCALE)).astype(np.float16)
    xr = xh.reshape(B, MIN, 128, D)                      # [b, m, j, ch]
    xt = xr.transpose(3, 2, 1, 0)                        # [ch, j, m, b]
    xt = np.ascontiguousarray(xt).reshape(D, 128, XW)    # [ch, j, m*4+b]

    fs = (filt * np.float32(SCALE)).astype(np.float16)   # [512, D]
    pf = np.zeros((D, 767), np.float16)
    pf[:, 127:639] = fs.T
    jj = np.arange(128)
    uu = np.arange(BW)
    idx = 127 - jj[:, None] + uu[None, :]                # [128, 640] in [0, 767)
    band = pf[:, idx]                                    # [ch, j, u]

    xg = xt.reshape(NCORES, NG, GCH, 128, XW)
    bg = band.reshape(NCORES, NG, GCH, 128, BW)
    slab = np.empty((NCORES, NG, 128, SLABW), np.float16)
    slab[:, :, :, : GCH * XW] = (
        xg.transpose(0, 1, 3, 2, 4).reshape(NCORES, NG, 128, GCH * XW)
    )
    slab[:, :, :, GCH * XW :] = (
        bg.transpose(0, 1, 3, 2, 4).reshape(NCORES, NG, 128, GCH * BW)
    )
    return slab


def kernel(x, last_input_init, filt):
    global LAST_RESULTS
    if "nc" not in _CACHE:
        _CACHE["nc"] = _build_bass()
    nc = _CACHE["nc"]

    slab = _prep_inputs(x, last_input_init, filt)
    in_maps = [{"slab": slab[core]} for core in range(NCORES)]

    trace = bool(os.environ.get("BASS_TRACE"))
    res = run_bass_kernel_spmd(nc, in_maps, list(range(NCORES)), trace=trace)
    LAST_RESULTS = res

    outs = []
    for core in range(NCORES):
        o = res.results[core]["out"].astype(np.float32)  # [NG, 128, GCH, 128]
        o = o.reshape(NG, 128, GCH, MOUT, B)             # [g, i, c, m0, b]
        o = o.transpose(4, 3, 1, 0, 2)                   # [b, m0, i, g, c]
        outs.append(o.reshape(B, L, DSH))
    out = np.concatenate(outs, axis=2)
    return np.ascontiguousarray(out, dtype=np.float32)


# revision 39
# speedup vs baseline: 1.6002x; 1.0864x over previous
"""Trainium2 Bass kernel for nn_CConv (causal depthwise FFT-conv, 512 taps).

The reference's FFT conv is exactly a causal depthwise conv1d with a
512-step learned init state prepended:
    out[b,t,c] = sum_k f[k,c] * xc[b, t+512-k, c],  xc = concat(init, x)

Mapping to the tensor engine (per channel c):
    out[i, (m0,b)] = sum_q  W_q[j,i].T @ X[j, (m0+q, b)]
where W_q[j,i] = f[i-j+128*(4-q), c] are 5 Toeplitz slices of a
[128 x 640] band built on the host, and X is the natural time-tiled
x (partition = t%128, columns = (time chunk, batch)).  The 5 matmuls
accumulate in PSUM.  Channels are sharded 8 ways across cores; inside a
core, channels stream in groups of 8 (fused x+band slab DMA -> 40
matmuls -> PSUM->SBUF cast -> batched output DMA).

HBM traffic is the bottleneck (the profile shows the DMA stream at
~355 GB/s, 99% of the per-core HBM roofline), so: operands are fp16
(filter prescaled by 32, x by 1/32 — exact powers of two keeping both
far from fp16 subnormals; fp32 PSUM accumulation) and the output is
written fp16 and upcast on host.  All DMAs are contiguous host-
prearranged slabs; output DMAs are batched 4 groups at a time with
smaller final batches so the kernel tail isn't gated on a large DMA.

Variants tried and rejected (kept out of the final kernel):
- on-chip Toeplitz expansion via SWDGE diagonal-AP DMAs: correct but
  SWDGE per-DMA overhead + single-port source reads made it slower;
- shipping only the nonzero 32-row-block rectangles of the band into a
  persistent zeroed ring: the ring memsets and quarter-partition DMAs
  cost more than the 3MB of HBM they saved;
- fp32/float32r matmuls (4x/4x cycles per row at N=128), TB=64 tiling
  (halves band bytes but doubles PE streaming time), FFT-by-matmul
  (4-8x the FLOPs).
"""

import os

import numpy as np

import concourse.bacc as bacc
import concourse.mybir as mybir
from concourse.bass_utils import run_bass_kernel_spmd
from concourse.tile import TileContext

B, L, D, CLEN = 4, 4096, 1024, 512
NCORES = 8
DSH = D // NCORES            # 128 channels per core
GCH = 8                      # channels per group
NG = DSH // GCH              # 16 groups per core
MIN = (CLEN + L) // 128      # 36 input chunks (4 init + 32 x)
MOUT = L // 128              # 32 output blocks
NQ = 5                       # contraction chunks per output block
XW = MIN * B                 # 144 x columns per channel
BW = 640                     # band columns per channel
CW = XW + BW                 # 784 slab columns per channel (x | band)
SCALE = 32.0

_CACHE = {}
LAST_RESULTS = None          # BassKernelResults of the most recent run


def _build_bass():
    # Bacc (not plain Bass): its compile() legalizes sync waits (>1 wait per
    # instruction gets split into InstEventSemaphore), which walrus requires.
    nc = bacc.Bacc(None, target_bir_lowering=False)
    f16, f32 = mybir.dt.float16, mybir.dt.float32
    sd = nc.declare_dram_parameter("slab", [NG, 128, GCH, CW], f16, isOutput=False)
    od = nc.declare_dram_parameter("out", [NG, 128, GCH, 128], f16, isOutput=True)

    with TileContext(nc) as tc:
        with (
            tc.tile_pool(name="sp", bufs=4) as sp,
            tc.tile_pool(name="op", bufs=4) as op,
            tc.tile_pool(name="pp", bufs=4, space="PSUM") as pp,
        ):
            batches = [(0, 4), (4, 4), (8, 4), (12, 2), (14, 1), (15, 1)]
            for g0, nb in batches:
                ot = op.tile([128, nb, GCH, 128], f16)
                for k in range(nb):
                    g = g0 + k
                    st = sp.tile([128, GCH, CW], f16)
                    if g == 0:
                        # split so the first channels' matmuls start sooner
                        nc.sync.dma_start(out=st[:, :2], in_=sd[g, :, :2])
                        nc.sync.dma_start(out=st[:, 2:], in_=sd[g, :, 2:])
                    else:
                        nc.sync.dma_start(out=st[:], in_=sd[g])
                    for c in range(GCH):
                        ps = pp.tile([128, 128], f32)
                        for q in range(NQ):
                            nc.tensor.matmul(
                                ps[:],
                                lhsT=st[:, c, XW + 128 * (4 - q) : XW + 128 * (5 - q)],
                                rhs=st[:, c, 4 * q : 4 * q + 128],
                                start=(q == 0),
                                stop=(q == NQ - 1),
                            )
                        nc.vector.tensor_copy(out=ot[:, k, c, :], in_=ps[:])
                nc.sync.dma_start(
                    out=od[g0 : g0 + nb].transpose([1, 0, 2, 3]), in_=ot[:]
                )
    nc.finalize()  # Bacc.compile(): reg alloc + sync-wait legalization
    return nc


def _prep_inputs(x, last_input_init, filt):
    """Host-side: cast/scale to fp16 and prearrange into the exact SBUF
    layout so every DMA is a contiguous line-rate copy.

    slab[core, g, j, c, 0:144]   = xc[b, 128*m + j, ch] / 32   at col m*4+b
    slab[core, g, j, c, 144:784] = 32*f[u - j, ch]             at col 144+u
    """
    x = np.asarray(x, dtype=np.float32)
    init = np.asarray(last_input_init, dtype=np.float32)
    filt = np.asarray(filt, dtype=np.float32)

    xc = np.concatenate(
        [np.broadcast_to(init[None], (B, CLEN, D)), x], axis=1
    )  # [B, 4608, D]
    xh = (xc * np.float32(1.0 / SCALE)).astype(np.float16)
    xr = xh.reshape(B, MIN, 128, D)                      # [b, m, j, ch]
    xt = xr.transpose(3, 2, 1, 0)                        # [ch, j, m, b]
    xt = np.ascontiguousarray(xt).reshape(D, 128, XW)    # [ch, j, m*4+b]

    fs = (filt * np.float32(SCALE)).astype(np.float16)   # [512, D]
    pf = np.zeros((D, 767), np.float16)
    pf[:, 127:639] = fs.T
    jj = np.arange(128)
    uu = np.arange(BW)
    idx = 127 - jj[:, None] + uu[None, :]                # [128, 640] in [0, 767)
    band = pf[:, idx]                                    # [ch, j, u]

    slab = np.empty((D, 128, CW), np.float16)
    slab[:, :, :XW] = xt
    slab[:, :, XW:] = band
    slab = slab.reshape(NCORES, NG, GCH, 128, CW).transpose(0, 1, 3, 2, 4)
    return np.ascontiguousarray(slab)


def kernel(x, last_input_init, filt):
    global LAST_RESULTS
    if "nc" not in _CACHE:
        _CACHE["nc"] = _build_bass()
    nc = _CACHE["nc"]

    slab = _prep_inputs(x, last_input_init, filt)
    in_maps = [{"slab": slab[core]} for core in range(NCORES)]

    trace = bool(os.environ.get("BASS_TRACE"))
    res = run_bass_kernel_spmd(nc, in_maps, list(range(NCORES)), trace=trace)
    LAST_RESULTS = res

    outs = []
    for core in range(NCORES):
        o = res.results[core]["out"].astype(np.float32)  # [NG, 128, GCH, 128]
        o = o.reshape(NG, 128, GCH, MOUT, B)             # [g, i, c, m0, b]
        o = o.transpose(4, 3, 1, 0, 2)                   # [b, m0, i, g, c]
        outs.append(o.reshape(B, L, DSH))
    out = np.concatenate(outs, axis=2)
    return np.ascontiguousarray(out, dtype=np.float32)


# revision 44
# speedup vs baseline: 1.6956x; 1.0596x over previous
"""Trainium2 Bass kernel for nn_CConv (causal depthwise FFT-conv, 512 taps).

The reference's FFT conv is exactly a causal depthwise conv1d with a
512-step learned init state prepended:
    out[b,t,c] = sum_k f[k,c] * xc[b, t+512-k, c],  xc = concat(init, x)

Mapping to the tensor engine (per channel c):
    out[i, (m0,b)] = sum_q  W_q[j,i].T @ X[j, (m0+q, b)]
where W_q[j,i] = f[i-j+128*(4-q), c] are 5 Toeplitz slices of a
[128 x 640] band built on the host, and X is the natural time-tiled
x (partition = t%128, columns = (time chunk, batch)).  The 5 matmuls
accumulate in PSUM.  Channels are sharded 8 ways across cores; inside a
core, channels stream in groups of 8 (fused x+band slab DMA -> 40
matmuls -> PSUM->SBUF cast -> batched output DMA).

HBM traffic is the bottleneck (the profile shows the DMA stream at
~355 GB/s, 99% of the per-core HBM roofline), so: operands are fp16
(filter prescaled by 32, x by 1/32 — exact powers of two keeping both
far from fp16 subnormals; fp32 PSUM accumulation) and the output is
written fp16 and upcast on host.  All DMAs are contiguous host-
prearranged slabs; output DMAs are batched 4 groups at a time with
smaller final batches so the kernel tail isn't gated on a large DMA.

Variants tried and rejected (kept out of the final kernel):
- on-chip Toeplitz expansion via SWDGE diagonal-AP DMAs: correct but
  SWDGE per-DMA overhead + single-port source reads made it slower;
- shipping only the nonzero 32-row-block rectangles of the band into a
  persistent zeroed ring: the ring memsets and quarter-partition DMAs
  cost more than the 3MB of HBM they saved;
- fp32/float32r matmuls (4x/4x cycles per row at N=128), TB=64 tiling
  (halves band bytes but doubles PE streaming time), FFT-by-matmul
  (4-8x the FLOPs).
"""

import os

import numpy as np

import concourse.bacc as bacc
import concourse.mybir as mybir
from concourse.bass_utils import run_bass_kernel_spmd
from concourse.tile import TileContext

B, L, D, CLEN = 4, 4096, 1024, 512
NCORES = 8
DSH = D // NCORES            # 128 channels per core
GCH = 8                      # channels per group
NG = DSH // GCH              # 16 groups per core
MIN = (CLEN + L) // 128      # 36 input chunks (4 init + 32 x)
MOUT = L // 128              # 32 output blocks
NQ = 5                       # contraction chunks per output block
XW = MIN * B                 # 144 x columns per channel
BW = 640                     # band columns per channel
CW = XW + BW                 # 784 slab columns per channel (x | band)
SCALE = 32.0

_CACHE = {}
LAST_RESULTS = None          # BassKernelResults of the most recent run


def _build_bass():
    # Bacc (not plain Bass): its compile() legalizes sync waits (>1 wait per
    # instruction gets split into InstEventSemaphore), which walrus requires.
    nc = bacc.Bacc(None, target_bir_lowering=False)
    f16, f32 = mybir.dt.float16, mybir.dt.float32
    sd = nc.declare_dram_parameter("slab", [NG, 128, GCH, CW], f16, isOutput=False)
    od = nc.declare_dram_parameter("out", [NG, 128, GCH, 128], f16, isOutput=True)

    with TileContext(nc) as tc:
        with (
            tc.tile_pool(name="sp", bufs=5) as sp,
            tc.tile_pool(name="op", bufs=4) as op,
            tc.tile_pool(name="pp", bufs=4, space="PSUM") as pp,
        ):
            batches = [(0, 4), (4, 4), (8, 4), (12, 2), (14, 1), (15, 1)]
            for g0, nb in batches:
                ot = op.tile([128, nb, GCH, 128], f16)
                for k in range(nb):
                    g = g0 + k
                    st = sp.tile([128, GCH, CW], f16)
                    if g == 0:
                        # split so the first channel's matmuls start sooner
                        nc.sync.dma_start(out=st[:, :1], in_=sd[g, :, :1])
                        nc.sync.dma_start(out=st[:, 1:], in_=sd[g, :, 1:])
                    else:
                        nc.sync.dma_start(out=st[:], in_=sd[g])
                    for c in range(GCH):
                        ps = pp.tile([128, 128], f32)
                        for q in range(NQ):
                            nc.tensor.matmul(
                                ps[:],
                                lhsT=st[:, c, XW + 128 * (4 - q) : XW + 128 * (5 - q)],
                                rhs=st[:, c, 4 * q : 4 * q + 128],
                                start=(q == 0),
                                stop=(q == NQ - 1),
                            )
                        nc.vector.tensor_copy(out=ot[:, k, c, :], in_=ps[:])
                # scalar engine = second HWDGE ring: output packets interleave
                # with the slab stream instead of queuing behind it
                nc.scalar.dma_start(
                    out=od[g0 : g0 + nb].transpose([1, 0, 2, 3]), in_=ot[:]
                )
    nc.finalize()  # Bacc.compile(): reg alloc + sync-wait legalization
    return nc


def _prep_inputs(x, last_input_init, filt):
    """Host-side: cast/scale to fp16 and prearrange into the exact SBUF
    layout so every DMA is a contiguous line-rate copy.

    slab[core, g, j, c, 0:144]   = xc[b, 128*m + j, ch] / 32   at col m*4+b
    slab[core, g, j, c, 144:784] = 32*f[u - j, ch]             at col 144+u
    """
    x = np.asarray(x, dtype=np.float32)
    init = np.asarray(last_input_init, dtype=np.float32)
    filt = np.asarray(filt, dtype=np.float32)

    xc = np.concatenate(
        [np.broadcast_to(init[None], (B, CLEN, D)), x], axis=1
    )  # [B, 4608, D]
    xh = (xc * np.float32(1.0 / SCALE)).astype(np.float16)
    xr = xh.reshape(B, MIN, 128, D)                      # [b, m, j, ch]
    xt = xr.transpose(3, 2, 1, 0)                        # [ch, j, m, b]
    xt = np.ascontiguousarray(xt).reshape(D, 128, XW)    # [ch, j, m*4+b]

    fs = (filt * np.float32(SCALE)).astype(np.float16)   # [512, D]
    pf = np.zeros((D, 767), np.float16)
    pf[:, 127:639] = fs.T
    jj = np.arange(128)
    uu = np.arange(BW)
    idx = 127 - jj[:, None] + uu[None, :]                # [128, 640] in [0, 767)
    band = pf[:, idx]                                    # [ch, j, u]

    slab = np.empty((D, 128, CW), np.float16)
    slab[:, :, :XW] = xt
    slab[:, :, XW:] = band
    slab = slab.reshape(NCORES, NG, GCH, 128, CW).transpose(0, 1, 3, 2, 4)
    return np.ascontiguousarray(slab)


def kernel(x, last_input_init, filt):
    global LAST_RESULTS
    if "nc" not in _CACHE:
        _CACHE["nc"] = _build_bass()
    nc = _CACHE["nc"]

    slab = _prep_inputs(x, last_input_init, filt)
    in_maps = [{"slab": slab[core]} for core in range(NCORES)]

    trace = bool(os.environ.get("BASS_TRACE"))
    res = run_bass_kernel_spmd(nc, in_maps, list(range(NCORES)), trace=trace)
    LAST_RESULTS = res

    outs = []
    for core in range(NCORES):
        o = res.results[core]["out"].astype(np.float32)  # [NG, 128, GCH, 128]
        o = o.reshape(NG, 128, GCH, MOUT, B)             # [g, i, c, m0, b]
        o = o.transpose(4, 3, 1, 0, 2)                   # [b, m0, i, g, c]
        outs.append(o.reshape(B, L, DSH))
    out = np.concatenate(outs, axis=2)
    return np.ascontiguousarray(out, dtype=np.float32)
